# revision 1
# baseline (speedup 1.0000x reference)
"""nn_CrossAttention — Trainium2 Bass kernel (8 NeuronCores, SPMD).

Sharding: core c handles batch b=c//2 and head-group g=c%2 (4 of 8 heads):
data-parallel over batch, tensor-parallel over heads. Each core computes
yT_partial = (softmax(Q_g K_g^T / sqrt(d)) V_g @ Wo_g)^T for its batch.
Host-side unshard sums the two head-group partials per batch, transposes,
and adds the output bias.

On-device layout: everything is computed in the "transposed" domain
(queries on the free dim) so the PE contracts along partitions without any
on-device transposes. Softmax denominators come free from an extra ones
column appended to V (row 64 of the O-matmul PSUM accumulator); the skipped
max-subtraction is safe because scores are O(1) for these inputs. Matmul
operands are float32r (full PE rate at moving-dim 512, ~1e-4 rel err).
"""

from contextlib import ExitStack

import numpy as np

import concourse.bass as bass
import concourse.mybir as mybir
import concourse.tile as tile
from concourse import bacc
from concourse.bass_utils import run_bass_kernel_spmd

F32 = mybir.dt.float32
F32R = mybir.dt.float32r

B, H = 4, 8
N = 4096          # queries per batch
M = 1024          # keys
QD = 1024         # query dim
CD = 768          # context dim
DH = 64           # head dim
HL = 4            # heads per core
IL = HL * DH      # local inner = 256
SCALE = DH ** -0.5

QCH = 512         # query chunk (moving dim)
NCH = N // QCH
NKT = M // 128
NQT = QD // 128
NCT = CD // 128
NYT = QD // 128


def _build(dt_mm=F32R, dt_pt=F32R):
    nc = bacc.Bacc("TRN2", target_bir_lowering=False, debug=False)

    xT = nc.declare_dram_parameter("xT", [QD, N], dt_mm, isOutput=False)
    ctxT = nc.declare_dram_parameter("ctxT", [CD, M], dt_mm, isOutput=False)
    wq = nc.declare_dram_parameter("wq", [QD, IL], dt_mm, isOutput=False)
    wk = nc.declare_dram_parameter("wk", [CD, IL], dt_mm, isOutput=False)
    wv = nc.declare_dram_parameter("wv", [CD, IL], dt_mm, isOutput=False)
    wo = nc.declare_dram_parameter("wo", [IL, QD], dt_mm, isOutput=False)
    yT = nc.declare_dram_parameter("yT", [QD, N], F32, isOutput=True)

    xT_r = xT.rearrange("(kt p) (c q) -> p kt c q", p=128, q=QCH)
    ctx_r = ctxT.rearrange("(ct p) m -> p ct m", p=128)
    wq_r = wq.rearrange("(kt p) i -> p kt i", p=128)
    wk_r = wk.rearrange("(ct p) i -> p ct i", p=128)
    wv_r = wv.rearrange("(ct p) i -> p ct i", p=128)
    wo_r = wo.rearrange("(it p) d -> p it d", p=128)
    yT_r = yT.rearrange("(yt p) (c q) -> p yt c q", p=128, q=QCH)

    with tile.TileContext(nc) as tc, ExitStack() as stack:
        sing = stack.enter_context(tc.tile_pool(name="sing", bufs=1))

        # ---- stage A: load weights, compute K^T and V_aug ----
        wq_sb = sing.tile([128, NQT, IL], dt_mm)
        nc.sync.dma_start(out=wq_sb, in_=wq_r)
        wo_sb = sing.tile([128, 2, QD], dt_mm)
        nc.sync.dma_start(out=wo_sb, in_=wo_r)
        kt_sb = sing.tile([128, 2, M], dt_mm)               # [inner%128, mi, keys]
        vaug_sb = sing.tile([128, NKT, HL, DH + 1], dt_pt)  # [key%128, kt, head, dh+1]
        nc.vector.memset(vaug_sb[:, :, :, DH:DH + 1].bitcast(F32), 1.0)

        with tc.tile_pool(name="stagea", bufs=1) as stagea, \
             tc.tile_pool(name="psa_a", bufs=2, space="PSUM") as psa_a:
            wk_sb = stagea.tile([128, NCT, IL], dt_mm)
            nc.sync.dma_start(out=wk_sb, in_=wk_r)
            wv_sb = stagea.tile([128, NCT, IL], dt_mm)
            nc.sync.dma_start(out=wv_sb, in_=wv_r)
            ctx_sb = stagea.tile([128, NCT, M], dt_mm)
            nc.sync.dma_start(out=ctx_sb, in_=ctx_r)

            for mi in range(2):
                for nch2 in range(M // QCH):
                    pk = psa_a.tile([128, QCH], F32, tag="ps1")
                    for ct in range(NCT):
                        nc.tensor.matmul(
                            pk, wk_sb[:, ct, mi * 128:(mi + 1) * 128],
                            ctx_sb[:, ct, nch2 * QCH:(nch2 + 1) * QCH],
                            start=(ct == 0), stop=(ct == NCT - 1))
                    nc.vector.tensor_copy(
                        kt_sb[:, mi, nch2 * QCH:(nch2 + 1) * QCH], pk)
            for kt in range(NKT):
                pv = psa_a.tile([128, IL], F32, tag="ps1")
                for ct in range(NCT):
                    nc.tensor.matmul(
                        pv, ctx_sb[:, ct, kt * 128:(kt + 1) * 128],
                        wv_sb[:, ct, :],
                        start=(ct == 0), stop=(ct == NCT - 1))
                nc.vector.tensor_copy(
                    vaug_sb[:, kt, :, 0:DH],
                    pv.rearrange("p (h d) -> p h d", h=HL))

        # ---- stage B pools (opened after stage A space is released) ----
        xpool = stack.enter_context(tc.tile_pool(name="xpool", bufs=2))
        qtp = stack.enter_context(tc.tile_pool(name="qtp", bufs=2))
        ptp = stack.enter_context(tc.tile_pool(name="ptp", bufs=2))
        o2p = stack.enter_context(tc.tile_pool(name="o2p", bufs=3))
        ypool = stack.enter_context(tc.tile_pool(name="ypool", bufs=2))
        smallp = stack.enter_context(tc.tile_pool(name="smallp", bufs=2))
        psq = stack.enter_context(tc.tile_pool(name="psq", bufs=1, space="PSUM"))
        pss = stack.enter_context(tc.tile_pool(name="pss", bufs=2, space="PSUM"))
        pso = stack.enter_context(tc.tile_pool(name="pso", bufs=1, space="PSUM"))
        psy = stack.enter_context(tc.tile_pool(name="psy", bufs=1, space="PSUM"))

        for c in range(NCH):
            xc = xpool.tile([128, NQT, QCH], dt_mm)
            nc.sync.dma_start(out=xc, in_=xT_r[:, :, c, :])

            qt = qtp.tile([128, 2, QCH], dt_mm)
            for mi in range(2):
                pq = psq.tile([128, QCH], F32, tag="ps1")
                for kt in range(NQT):
                    nc.tensor.matmul(
                        pq, wq_sb[:, kt, mi * 128:(mi + 1) * 128],
                        xc[:, kt, :],
                        start=(kt == 0), stop=(kt == NQT - 1))
                nc.vector.tensor_copy(qt[:, mi, :], pq)

            o2t = []
            for p in range(2):   # head pairs: heads (2p, 2p+1)
                ptA = ptp.tile([128, NKT, QCH], dt_pt, tag="ptA")
                ptB = ptp.tile([128, NKT, QCH], dt_pt, tag="ptB")
                for kt in range(NKT):
                    psa = pss.tile([128, QCH], F32, tag="psa")
                    psb = pss.tile([128, QCH], F32, tag="psb")
                    nc.tensor.matmul(
                        psa, kt_sb[0:64, p, kt * 128:(kt + 1) * 128],
                        qt[0:64, p, :], start=True, stop=True,
                        tile_position=(0, 0))
                    nc.tensor.matmul(
                        psb, kt_sb[64:128, p, kt * 128:(kt + 1) * 128],
                        qt[64:128, p, :], start=True, stop=True,
                        tile_position=(64, 0))
                    nc.scalar.activation(ptA[:, kt, :], psa,
                                         mybir.ActivationFunctionType.Exp,
                                         scale=SCALE)
                    nc.scalar.activation(ptB[:, kt, :], psb,
                                         mybir.ActivationFunctionType.Exp,
                                         scale=SCALE)
                poa = pso.tile([DH + 1, QCH], F32, tag="poa")
                pob = pso.tile([DH + 1, QCH], F32, tag="pob")
                for kt in range(NKT):
                    nc.tensor.matmul(poa, vaug_sb[:, kt, 2 * p, :],
                                     ptA[:, kt, :],
                                     start=(kt == 0), stop=(kt == NKT - 1))
                    nc.tensor.matmul(pob, vaug_sb[:, kt, 2 * p + 1, :],
                                     ptB[:, kt, :],
                                     start=(kt == 0), stop=(kt == NKT - 1))
                ra = smallp.tile([1, QCH], F32, tag="ra")
                rb = smallp.tile([1, QCH], F32, tag="rb")
                nc.vector.reciprocal(ra, poa[DH:DH + 1, :])
                nc.vector.reciprocal(rb, pob[DH:DH + 1, :])
                bca = smallp.tile([64, QCH], F32, tag="bca")
                bcb = smallp.tile([64, QCH], F32, tag="bcb")
                nc.gpsimd.partition_broadcast(bca, ra)
                nc.gpsimd.partition_broadcast(bcb, rb)
                ot = o2p.tile([128, QCH], dt_mm)
                nc.vector.tensor_mul(ot[0:64, :], poa[0:DH, :], bca)
                nc.vector.tensor_mul(ot[64:128, :], pob[0:DH, :], bcb)
                o2t.append(ot)

            yc = ypool.tile([128, NYT, QCH], F32)
            for yt in range(NYT):
                py = psy.tile([128, QCH], F32)
                nc.tensor.matmul(py, wo_sb[:, 0, yt * 128:(yt + 1) * 128],
                                 o2t[0], start=True, stop=False)
                nc.tensor.matmul(py, wo_sb[:, 1, yt * 128:(yt + 1) * 128],
                                 o2t[1], start=False, stop=True)
                nc.vector.tensor_copy(yc[:, yt, :], py)
            nc.sync.dma_start(out=yT_r[:, :, c, :], in_=yc)

    nc.compile()
    return nc


_NC_CACHE = {}


def _get_nc():
    if "nc" not in _NC_CACHE:
        _NC_CACHE["nc"] = _build()
    return _NC_CACHE["nc"]


def kernel(x, context, Wq, Wk, Wv, Wo, bo):
    x = np.asarray(x, np.float32)
    context = np.asarray(context, np.float32)
    Wq = np.asarray(Wq, np.float32)
    Wk = np.asarray(Wk, np.float32)
    Wv = np.asarray(Wv, np.float32)
    Wo = np.asarray(Wo, np.float32)
    bo = np.asarray(bo, np.float32)

    nc = _get_nc()
    in_maps = []
    for c in range(8):
        b, g = c // 2, c % 2
        sl = slice(g * IL, (g + 1) * IL)
        in_maps.append({
            "xT": np.ascontiguousarray(x[b].T),
            "ctxT": np.ascontiguousarray(context[b].T),
            "wq": np.ascontiguousarray(Wq[:, sl]),
            "wk": np.ascontiguousarray(Wk[:, sl]),
            "wv": np.ascontiguousarray(Wv[:, sl]),
            "wo": np.ascontiguousarray(Wo[sl, :]),
        })

    res = None
    for attempt in range(3):
        try:
            res = run_bass_kernel_spmd(nc, in_maps, core_ids=list(range(8)))
            break
        except Exception:
            # the axon-tunneled device occasionally reports
            # NRT_EXEC_UNIT_UNRECOVERABLE; the failure sticks to the PJRT
            # client, so tear down the backend to get a fresh worker
            if attempt == 2:
                raise
            import time
            import jax
            time.sleep(10)
            try:
                jax.clear_caches()
                jax.extend.backend.clear_backends()
            except Exception:
                pass
    ys = []
    for b in range(B):
        yt = res.results[2 * b]["yT"] + res.results[2 * b + 1]["yT"]
        ys.append(yt.T + bo[None, :])
    return np.stack(ys, 0).astype(np.float32)

